# revision 12
# baseline (speedup 1.0000x reference)
"""Multi-head attention (B=2, S=2048, D=1024, H=16) on 8 TRN2 NeuronCores.

Sharding: data-parallel over batch (2 groups of 4 cores), tensor-parallel over
heads (4 heads per core). W_o is row-parallel; the partial outputs are summed
on host at unshard time (the all-reduce equivalent).

Per-core kernel (all matmuls bf16 with fp32 PSUM accumulation, transposed
orientation so no on-chip transposes are needed):
  QT = (Wq_loc/8) @ x.T          [256, 2048]   (d_local on partitions)
  KT = Wk_loc @ x.T              [256, 2048]
  V  = x @ Wv_loc.T              [2048, 256]   (seq on partitions)
  per (q-tile 512, k-tile 128):
    scoresT[k, q] = KT_chunk.T @ QT  (2-head row-packed matmuls, d=64)
    attn = exp(scoresT) * exp(mask).T    (ScalarE exp, VectorE masked mul)
    pv[h] += V_chunk.T @ attn        (2-head col-packed, accumulated in PSUM)
    den[h] += ones.T @ attn          (softmax denominators, 4-head col-packed)
  outT[h] = pv[h] / den[h]         (reciprocal + PE broadcast + mul)
  yT_partial = WoT_loc.T @ outT    [1024, 2048] fp32 -> DRAM
"""

import numpy as np
import ml_dtypes

B = 2
S = 2048
D = 1024
H = 16
DK = 64
HPC = 4          # heads per core
DL = HPC * DK    # 256 local head dims
N_CORES = 8
QT_TILES = 4     # q tiles of 512
QTS = 512
KC = 16          # k chunks of 128
DC = 8           # D chunks of 128

_BF16 = ml_dtypes.bfloat16

_COMPILED = {}


def _build_program():
    import concourse.bass as bass
    import concourse.mybir as mybir
    import concourse.tile as tile
    from concourse import bacc

    f32 = mybir.dt.float32
    bf16 = mybir.dt.bfloat16

    nc = bacc.Bacc("TRN2", target_bir_lowering=False, debug=False,
                   num_devices=N_CORES)

    x_t = nc.dram_tensor("x_t", [D, S], bf16, kind="ExternalInput").ap()
    wq_t = nc.dram_tensor("wq_t", [D, DL], bf16, kind="ExternalInput").ap()
    wk_t = nc.dram_tensor("wk_t", [D, DL], bf16, kind="ExternalInput").ap()
    wv_t = nc.dram_tensor("wv_t", [D, DL], bf16, kind="ExternalInput").ap()
    wo_t = nc.dram_tensor("wo_t", [DL, D], bf16, kind="ExternalInput").ap()
    maskexp_t = nc.dram_tensor("maskexp_t", [S, S], bf16,
                               kind="ExternalInput").ap()
    blockones = nc.dram_tensor("blockones", [128, 256], f32,
                               kind="ExternalInput").ap()
    ones_col = nc.dram_tensor("ones_col", [128, 32], bf16,
                              kind="ExternalInput").ap()
    y_t = nc.dram_tensor("y_t", [D, S], f32, kind="ExternalOutput").ap()

    with tile.TileContext(nc) as tc:
        with (
            tc.tile_pool(name="consts", bufs=1) as consts,
            tc.tile_pool(name="acts", bufs=1) as acts,
            tc.tile_pool(name="mask", bufs=3) as maskp,
            tc.tile_pool(name="attn", bufs=3) as attnp,
            tc.tile_pool(name="norm", bufs=2) as normp,
            tc.tile_pool(name="yev", bufs=2) as yevp,
        ):
            # ---- load x first (phase 1 is blocked on it), then weights ----
            xt_sb = consts.tile([128, DC * S], bf16, tag="xt")
            wq_sb = consts.tile([128, DC * DL], bf16, tag="wq")
            wk_sb = consts.tile([128, DC * DL], bf16, tag="wk")
            wv_sb = consts.tile([128, DC * DL], bf16, tag="wv")
            for dc in range(DC):
                nc.sync.dma_start(xt_sb[:, dc * S:(dc + 1) * S],
                                  x_t[dc * 128:(dc + 1) * 128, :])
                nc.sync.dma_start(wq_sb[:, dc * DL:(dc + 1) * DL],
                                  wq_t[dc * 128:(dc + 1) * 128, :])
                nc.sync.dma_start(wk_sb[:, dc * DL:(dc + 1) * DL],
                                  wk_t[dc * 128:(dc + 1) * 128, :])
            for dc in range(DC):
                nc.sync.dma_start(wv_sb[:, dc * DL:(dc + 1) * DL],
                                  wv_t[dc * 128:(dc + 1) * 128, :])
            wo_sb = consts.tile([128, 2 * D], bf16, tag="wo")
            for dl in range(2):
                nc.sync.dma_start(wo_sb[:, dl * D:(dl + 1) * D],
                                  wo_t[dl * 128:(dl + 1) * 128, :])
            bones_sb = consts.tile([128, 256], f32, tag="bones")
            nc.sync.dma_start(bones_sb[:], blockones[:])
            ones_sb = consts.tile([128, 32], bf16, tag="ones")
            nc.sync.dma_start(ones_sb[:], ones_col[:])

            # ---- projections ----
            # QT / KT: [256, S] as two partition chunks (head pairs)
            proj_scope = tc.tile_pool(name="ps_proj", bufs=2, space="PSUM")
            ps_proj = proj_scope.__enter__()
            qt_sb = [acts.tile([128, S], bf16, tag=f"qt{dl}", name=f"qt{dl}") for dl in range(2)]
            kt_sb = [acts.tile([128, S], bf16, tag=f"kt{dl}", name=f"kt{dl}") for dl in range(2)]
            for (w_sb, dst) in ((wq_sb, qt_sb), (wk_sb, kt_sb)):
                for dl in range(2):
                    for st in range(8):
                        ps = ps_proj.tile([128, 256], f32, tag="proj")
                        for dc in range(DC):
                            nc.tensor.matmul(
                                ps[:],
                                lhsT=w_sb[:, dc * DL + dl * 128:
                                          dc * DL + (dl + 1) * 128],
                                rhs=xt_sb[:, dc * S + st * 256:
                                          dc * S + (st + 1) * 256],
                                start=(dc == 0), stop=(dc == DC - 1))
                        nc.scalar.copy(dst[dl][:, st * 256:(st + 1) * 256],
                                       ps[:])
            # V: [S, 256] natural layout, seq on partitions
            v_sb = acts.tile([128, KC * DL], bf16, tag="v")
            for sc in range(KC):
                ps = ps_proj.tile([128, DL], f32, tag="vproj")
                for dc in range(DC):
                    nc.tensor.matmul(
                        ps[:],
                        lhsT=xt_sb[:, dc * S + sc * 128:dc * S + (sc + 1) * 128],
                        rhs=wv_sb[:, dc * DL:(dc + 1) * DL],
                        start=(dc == 0), stop=(dc == DC - 1))
                nc.scalar.copy(v_sb[:, sc * DL:(sc + 1) * DL], ps[:])

            proj_scope.__exit__(None, None, None)

            # ---- attention + output projection, per q tile ----
            attn_scope1 = tc.tile_pool(name="ps_s", bufs=1, space="PSUM")
            attn_scope2 = tc.tile_pool(name="ps_pv", bufs=1, space="PSUM")
            attn_scope3 = tc.tile_pool(name="ps_small", bufs=1, space="PSUM")
            ps_s = attn_scope1.__enter__()
            ps_pv = attn_scope2.__enter__()
            ps_small = attn_scope3.__enter__()
            outT_sb = [acts.tile([128, S], bf16, tag=f"outT{i}", name=f"outT{i}")
                       for i in range(2)]
            def outproj(qt, j, pool=None, tag="small"):
                pool = pool or ps_small
                yp = pool.tile([128, QTS], f32, tag=tag, name="yp")
                for half in range(2):
                    sl = slice(qt * QTS + half * 256, qt * QTS + half * 256 + 256)
                    for dl in range(2):
                        nc.tensor.matmul(
                            yp[:, half * 256:half * 256 + 256],
                            lhsT=wo_sb[:, dl * D + j * 128:dl * D + (j + 1) * 128],
                            rhs=outT_sb[dl][:, sl],
                            start=(dl == 0), stop=(dl == 1))
                yev = yevp.tile([128, QTS], f32, tag="yev", name="yev")
                nc.vector.tensor_copy(yev[:], yp[:])
                nc.sync.dma_start(
                    y_t[j * 128:(j + 1) * 128, qt * QTS:(qt + 1) * QTS],
                    yev[:])

            for qt in range(QT_TILES):
                pv0 = ps_pv.tile([128, QTS], f32, tag="pv0")
                pv1 = ps_pv.tile([128, QTS], f32, tag="pv1")
                den = ps_pv.tile([128, QTS], f32, tag="den")
                for kc in range(KC):
                    # previous q-tile's output projection, spread across this
                    # loop so its PSUM-slot serialization hides under the
                    # ACT-bound pipeline
                    if qt > 0 and kc < DC:
                        outproj(qt - 1, kc)
                    mt = maskp.tile([128, QTS], bf16, tag="mask")
                    nc.sync.dma_start(
                        mt[:], maskexp_t[kc * 128:(kc + 1) * 128,
                                         qt * QTS:(qt + 1) * QTS])
                    # two half-tiles (heads 0,1 | heads 2,3) pipeline the
                    # PE->ACT->DVE->PE chain with only 4 PSUM banks
                    attn_tiles = []
                    for half in range(2):
                        scores = ps_s.tile([128, 2 * QTS], f32,
                                           tag=f"scores{half}",
                                           name=f"scores{half}")
                        for hh in range(2):
                            h = 2 * half + hh
                            hp, r0 = h // 2, 64 * (h % 2)
                            nc.tensor.matmul(
                                scores[:, hh * QTS:(hh + 1) * QTS],
                                lhsT=kt_sb[hp][r0:r0 + 64,
                                               kc * 128:(kc + 1) * 128],
                                rhs=qt_sb[hp][r0:r0 + 64,
                                              qt * QTS:(qt + 1) * QTS],
                                start=True, stop=True,
                                tile_position=(r0, 0))
                        attn_e = attnp.tile([128, 2 * QTS], bf16,
                                            tag=f"attn_e{half}",
                                            name=f"attn_e{half}")
                        nc.scalar.activation(attn_e[:], scores[:],
                                             mybir.ActivationFunctionType.Exp)
                        attn = attnp.tile([128, 2 * QTS], bf16,
                                          tag=f"attn{half}",
                                          name=f"attn{half}")
                        a3 = attn[:].rearrange("p (h m) -> p h m", h=2)
                        e3 = attn_e[:].rearrange("p (h m) -> p h m", h=2)
                        m3 = mt[:].unsqueeze(1).broadcast_to([128, 2, QTS])
                        nc.vector.tensor_mul(a3, e3, m3)
                        pv = pv0 if half == 0 else pv1
                        for hh in range(2):
                            h = 2 * half + hh
                            nc.tensor.matmul(
                                pv[64 * hh:64 * (hh + 1), :],
                                lhsT=v_sb[:, kc * DL + h * DK:
                                          kc * DL + (h + 1) * DK],
                                rhs=attn[:, hh * QTS:(hh + 1) * QTS],
                                start=(kc == 0), stop=(kc == KC - 1),
                                tile_position=(0, 64 * hh))
                        attn_tiles.append(attn)
                    # all 4 denominator matmuls together: disjoint 32-col
                    # groups -> 4-way in-array concurrency
                    for h in range(HPC):
                        nc.tensor.matmul(
                            den[32 * h:32 * (h + 1), :],
                            lhsT=ones_sb[:],
                            rhs=attn_tiles[h // 2][:, (h % 2) * QTS:
                                                   (h % 2 + 1) * QTS],
                            start=(kc == 0), stop=(kc == KC - 1),
                            tile_position=(0, 32 * h))
                # normalize: outT[64h:64h+64, q] = pv[h] * (1/den[h])
                denr = normp.tile([128, QTS], f32, tag="denr")
                nc.vector.reciprocal(denr[:], den[:])
                for i, pv in enumerate((pv0, pv1)):
                    bps = ps_small.tile([128, QTS], f32, tag="small")
                    nc.tensor.matmul(bps[:],
                                     lhsT=bones_sb[:, 128 * i:128 * (i + 1)],
                                     rhs=denr[:],
                                     start=True, stop=True)
                    bsb = normp.tile([128, QTS], f32, tag="bsb")
                    nc.vector.tensor_copy(bsb[:], bps[:])
                    nc.vector.tensor_mul(
                        outT_sb[i][:, qt * QTS:(qt + 1) * QTS], pv[:], bsb[:])
            for j in range(DC):
                # reuse the now-idle scores slots to double-buffer the tail
                outproj(QT_TILES - 1, j, pool=ps_s, tag=f"scores{j % 2}")
            attn_scope3.__exit__(None, None, None)
            attn_scope2.__exit__(None, None, None)
            attn_scope1.__exit__(None, None, None)

    nc.compile()
    return nc


def _get_program():
    if "nc" not in _COMPILED:
        _COMPILED["nc"] = _build_program()
    return _COMPILED["nc"]


def _make_in_maps(x, mask, W_q, W_k, W_v, W_o):
    x = np.asarray(x, dtype=np.float32)
    mask = np.asarray(mask, dtype=np.float32)
    W_q = np.asarray(W_q, dtype=np.float32)
    W_k = np.asarray(W_k, dtype=np.float32)
    W_v = np.asarray(W_v, dtype=np.float32)
    W_o = np.asarray(W_o, dtype=np.float32)

    scale = np.float32(1.0 / np.sqrt(DK))
    maskexp_t = np.exp(mask, dtype=np.float32).T.astype(_BF16)
    maskexp_t = np.ascontiguousarray(maskexp_t)
    # blockones[:, 128i:128(i+1)] is the selection matrix for pv bank i:
    # out rows 0-63 <- den head 2i (denc rows 32*2i), rows 64-127 <- head 2i+1
    blockones = np.zeros((128, 256), dtype=np.float32)
    blockones[0, 0:64] = 1.0
    blockones[32, 64:128] = 1.0
    blockones[64, 128 + 0:128 + 64] = 1.0
    blockones[96, 128 + 64:128 + 128] = 1.0
    ones_col = np.ones((128, 32), dtype=_BF16)

    in_maps = []
    for c in range(N_CORES):
        b, g = c // 4, c % 4
        rows = slice(g * DL, (g + 1) * DL)
        in_maps.append({
            "x_t": np.ascontiguousarray(x[b].T).astype(_BF16),
            "wq_t": np.ascontiguousarray((W_q[rows, :] * scale).T).astype(_BF16),
            "wk_t": np.ascontiguousarray(W_k[rows, :].T).astype(_BF16),
            "wv_t": np.ascontiguousarray(W_v[rows, :].T).astype(_BF16),
            "wo_t": np.ascontiguousarray(W_o[:, rows].T).astype(_BF16),
            "maskexp_t": maskexp_t,
            "blockones": blockones,
            "ones_col": ones_col,
        })
    return in_maps


def kernel(x, mask, W_q, W_k, W_v, W_o):
    from concourse.bass_utils import run_bass_kernel_spmd

    in_maps = _make_in_maps(x, mask, W_q, W_k, W_v, W_o)
    nc = _get_program()
    res = run_bass_kernel_spmd(nc, in_maps, core_ids=list(range(N_CORES)))

    y = np.zeros((B, S, D), dtype=np.float32)
    for c in range(N_CORES):
        y[c // 4] += res.results[c]["y_t"].T
    return y
